# revision 1
# baseline (speedup 1.0000x reference)
"""CPI_DGLLife kernel for 8 Trainium2 NeuronCores (SPMD).

GCN over a 65536-node graph + protein conv1d branch + CPI head.
Sharding: data-parallel over the 512-graph batch (64 graphs / core).
Each core: full h0 table build (replicated), dma_gather edge aggregation
for its dst nodes, fp32r conv stack for its 64 proteins.
"""
import sys
sys.path.insert(0, "/opt/trn_rl_repo")
import contextlib
import numpy as np

import concourse.bass as bass
import concourse.bacc as bacc
import concourse.tile as tile
from concourse import mybir
from concourse.bass_utils import run_bass_kernel_spmd
from concourse.masks import make_identity

dt = mybir.dt
AF = mybir.ActivationFunctionType
ALU = mybir.AluOpType
AX = mybir.AxisListType

P = 128
N, E, B, L = 65536, 262144, 512, 1000
IN_DIM, HID, VOCAB = 74, 128, 25
CHANNELS = [HID, 96, 128, IN_DIM, HID]
NCORES = 8
GPC = B // NCORES              # graphs per core = 64
PPC = GPC                      # proteins per core = 64
# h0 tables: 512-aligned split, local idx = node - base + 1, row 0 = zeros
TBL_BASES = [0, 32256, 64512]
TBL_NNODES = [32256, 32256, 1024]
TBL_ROWS = [n + 1 for n in TBL_NNODES]
TOK_BUDGET = 4096              # max tokens per dma_gather instruction
LCONV = 1002                   # 1000 + 2 guard cols


# ------------------------------------------------------------------ host prep
def _host_prep(inputs):
    graph_ids = np.asarray(inputs["graph_ids"])
    src = np.concatenate([np.asarray(inputs["edge_src"]).astype(np.int64),
                          np.arange(N, dtype=np.int64)])
    dst = np.concatenate([np.asarray(inputs["edge_dst"]).astype(np.int64),
                          np.arange(N, dtype=np.int64)])
    deg_out = np.bincount(src, minlength=N).astype(np.float32)
    deg_in = np.bincount(dst, minlength=N).astype(np.float32)

    core_node_lo = np.searchsorted(graph_ids, np.arange(0, B + 1, GPC))
    ncore_nodes = core_node_lo[1:] - core_node_lo[:-1]
    NT = int(np.ceil(ncore_nodes.max() / P))  # tiles per core (uniform)
    NPAD = NT * P

    # per-core degree-sorted node permutation (padded with -1)
    perm = np.full((NCORES, NPAD), -1, np.int64)
    for c in range(NCORES):
        lo, hi = int(core_node_lo[c]), int(core_node_lo[c + 1])
        order = np.argsort(-deg_in[lo:hi], kind="stable") + lo
        perm[c, :hi - lo] = order

    # deg_in per perm position (pad 1.0), laid out [P, NT] (p, t)
    deg_in_perm = np.ones((NCORES, NPAD), np.float32)
    m = perm >= 0
    deg_in_perm[m] = deg_in[perm[m]]
    deg_in_perm = deg_in_perm.reshape(NCORES, NT, P).transpose(0, 2, 1).copy()

    # S tiles: [NT, P, GPC] graph membership of permuted nodes
    S = np.zeros((NCORES, NT, P, GPC), np.float32)
    for c in range(NCORES):
        pm = perm[c]
        valid = pm >= 0
        g = graph_ids[pm[valid]] - c * GPC
        tt = np.arange(NPAD)[valid] // P
        pp = np.arange(NPAD)[valid] % P
        S[c, tt, pp, g] = 1.0

    # node -> (core, tile-position) in permuted order
    pos_of = np.full(N, -1, np.int64)
    core_of = np.full(N, -1, np.int64)
    for c in range(NCORES):
        pm = perm[c]
        v = pm >= 0
        pos_of[pm[v]] = np.arange(NPAD)[v]
        core_of[pm[v]] = c

    # table id + local row of each node (as gather source)
    tbl_of = np.digitize(np.arange(N), TBL_BASES[1:])
    loc_of = (np.arange(N) - np.asarray(TBL_BASES)[tbl_of] + 1).astype(np.int64)

    # edge placement: core/tile/lane from dst, table/local from src
    ec = core_of[dst]
    et = pos_of[dst] // P
    ep = pos_of[dst] % P
    etbl = tbl_of[src]
    eloc = loc_of[src]

    # slot index within (core, tile, lane, table) group
    key = (((ec * NT + et) * P + ep) * 3 + etbl)
    order = np.argsort(key, kind="stable")
    ks = key[order]
    starts = np.r_[0, np.flatnonzero(np.diff(ks)) + 1]
    grp_len = np.diff(np.r_[starts, E + N])
    slot_sorted = np.arange(E + N) - np.repeat(starts, grp_len)
    slot = np.empty(E + N, np.int64)
    slot[order] = slot_sorted
    # counts per (c, t, p, T) -> kmax per (t, T) across cores/lanes
    cnt = np.zeros(NCORES * NT * P * 3, np.int64)
    uk, uc = np.unique(ks, return_counts=True)
    cnt[uk] = uc
    cnt = cnt.reshape(NCORES, NT, P, 3)
    kmax = cnt.max(axis=2).max(axis=0)  # [NT, 3]

    # gather token schedule per table: tiles packed into instructions
    sched = []  # per table: list of instruction = list of (tile, k)
    for T in range(3):
        instrs, cur, tok = [], [], 0
        for t in range(NT):
            k = int(kmax[t, T])
            if k == 0:
                continue
            if tok + k * P > TOK_BUDGET and cur:
                instrs.append(cur)
                cur, tok = [], 0
            cur.append((t, k))
            tok += k * P
        if cur:
            instrs.append(cur)
        sched.append(instrs)

    # token offset of each tile inside its table stream
    tile_off = np.full((3, NT), 0, np.int64)
    tok_total = [0, 0, 0]
    for T in range(3):
        off = 0
        for ins in sched[T]:
            for (t, k) in ins:
                tile_off[T, t] = off
                off += k * P
        tok_total[T] = max(off, 128)

    idx_flat = [np.zeros((NCORES, tok_total[T]), np.int16) for T in range(3)]
    tok_pos = tile_off[etbl, et] + slot * P + ep
    for T in range(3):
        mT = etbl == T
        idx_flat[T][ec[mT], tok_pos[mT]] = eloc[mT].astype(np.int16)

    def wrap(a):  # token-major -> wrapped [128, tokens//16]
        ncol = a.shape[1] // 16
        w = a.reshape(a.shape[0], ncol, 16).transpose(0, 2, 1)
        return np.ascontiguousarray(np.tile(w, (1, 8, 1)))

    idx_wrapped = [wrap(ix) for ix in idx_flat]

    # per-token deg_out in gather-output layout [128, tokens//128]
    nf = np.asarray(inputs["node_feats"], np.float32)
    tabs = []
    dtok = []
    for T in range(3):
        tb = np.zeros((TBL_ROWS[T], P), np.float32)
        nn = TBL_NNODES[T]
        tb[1:1 + nn, :IN_DIM] = nf[TBL_BASES[T]:TBL_BASES[T] + nn]
        tabs.append(tb)
        d = np.ones((NCORES, tok_total[T]), np.float32)
        mT = etbl == T
        d[ec[mT], tok_pos[mT]] = deg_out[src[mT]]
        dtok.append(np.ascontiguousarray(
            d.reshape(NCORES, tok_total[T] // P, P).transpose(0, 2, 1)))

    # one-hot proteins grouped 4/DMA: [PPC//4, 128, LCONV], p = g*4+s
    seq = np.asarray(inputs["protein_seq"]).reshape(NCORES, PPC, L)
    oh = np.zeros((NCORES, PPC, 32, LCONV), np.float32)
    iot = np.arange(VOCAB)[None, None, :, None]
    oh[:, :, :VOCAB, 1:1 + L] = (seq[:, :, None, :] == iot)
    oh = np.ascontiguousarray(
        oh.reshape(NCORES, PPC // 4, 4 * 32, LCONV))

    shared = {
        "tab0": tabs[0], "tab1": tabs[1], "tab2": tabs[2],
        "W_gc": np.asarray(inputs["W_gc"], np.float32),
        "b_gc": np.asarray(inputs["b_gc"], np.float32).reshape(HID, 1),
        "W_ro_in": np.asarray(inputs["W_ro_in"], np.float32),
        "b_ro_in": np.asarray(inputs["b_ro_in"], np.float32).reshape(HID, 1),
        "W_ro_out": np.asarray(inputs["W_ro_out"], np.float32),
        "b_ro_out": np.asarray(inputs["b_ro_out"], np.float32).reshape(HID, 1),
        "Wc1": np.asarray(inputs["Wc1"], np.float32),
        "bc1": np.asarray(inputs["bc1"], np.float32).reshape(HID, 1),
        "Wc2": np.asarray(inputs["Wc2"], np.float32),
        "bc2": np.asarray(inputs["bc2"], np.float32).reshape(HID, 1),
        "embedT": np.ascontiguousarray(
            np.asarray(inputs["embed"], np.float32).T),       # [HID, 25]
        "Wf1_r": np.ascontiguousarray(
            np.asarray(inputs["Wf1"], np.float32).reshape(2, HID, 2 * HID)),
        "bf1_r": np.ascontiguousarray(
            np.asarray(inputs["bf1"], np.float32).reshape(2, HID, 1)),
        "Wf2_r": np.ascontiguousarray(
            np.asarray(inputs["Wf2"], np.float32).reshape(2, HID, 1)),
        "bf2": np.asarray(inputs["bf2"], np.float32).reshape(1, 1),
    }
    for l in range(4):
        K = np.asarray(inputs["K%d" % (l + 1)], np.float32)  # [o, i, 3]
        shared["K%dT" % (l + 1)] = np.ascontiguousarray(
            K.transpose(1, 2, 0))                            # [i, 3, o]
        shared["cb%d" % (l + 1)] = np.asarray(
            inputs["cb%d" % (l + 1)], np.float32).reshape(-1, 1)

    percore = []
    for c in range(NCORES):
        percore.append({
            "deg_in_perm": np.ascontiguousarray(deg_in_perm[c]),
            "S": np.ascontiguousarray(S[c]),
            "onehot": np.ascontiguousarray(oh[c]),
            "ix0": idx_wrapped[0][c],
            "ix1": idx_wrapped[1][c],
            "ix2": idx_wrapped[2][c],
            "dtok0": dtok[0][c], "dtok1": dtok[1][c], "dtok2": dtok[2][c],
        })
    meta = dict(NT=NT, sched=sched, tok_total=tok_total)
    return shared, percore, meta


# --------------------------------------------------------------- device build
def _build(shared, meta):
    NT = meta["NT"]
    sched = meta["sched"]
    tok_total = meta["tok_total"]

    nc = bacc.Bacc("TRN2", target_bir_lowering=False, debug=False,
                   num_devices=NCORES, num_swdge_queues=4)
    f32, f32r, i16 = dt.float32, dt.float32r, dt.int16

    D = {k: nc.dram_tensor(k, list(v.shape), dt.from_np(v.dtype),
                           kind="ExternalInput")
         for k, v in shared.items()}
    D["deg_in_perm"] = nc.dram_tensor("deg_in_perm", [P, NT], f32,
                                      kind="ExternalInput")
    D["S"] = nc.dram_tensor("S", [NT, P, GPC], f32, kind="ExternalInput")
    D["onehot"] = nc.dram_tensor("onehot", [PPC // 4, P, LCONV], f32,
                                 kind="ExternalInput")
    for T in range(3):
        D["ix%d" % T] = nc.dram_tensor("ix%d" % T, [P, tok_total[T] // 16],
                                       i16, kind="ExternalInput")
    tabs = [D["tab%d" % T] for T in range(3)]
    for T in range(3):
        D["dtok%d" % T] = nc.dram_tensor("dtok%d" % T, [P, tok_total[T] // P],
                                         f32, kind="ExternalInput")
    out_d = nc.dram_tensor("out", [1, GPC], f32, kind="ExternalOutput")

    with tile.TileContext(nc) as tc, contextlib.ExitStack() as ctx:
        wp = ctx.enter_context(tc.tile_pool(name="wp", bufs=1))
        h0p = ctx.enter_context(tc.tile_pool(name="h0p", bufs=3))
        gp = ctx.enter_context(tc.tile_pool(name="gp", bufs=1))
        accp = ctx.enter_context(tc.tile_pool(name="accp", bufs=1))
        cvp = ctx.enter_context(tc.tile_pool(name="cvp", bufs=2))
        gnp = ctx.enter_context(tc.tile_pool(name="gnp", bufs=3))
        pcv = ctx.enter_context(tc.tile_pool(name="pcv", bufs=4, space="PSUM"))
        pgn = ctx.enter_context(tc.tile_pool(name="pgn", bufs=2, space="PSUM"))
        ps1 = ctx.enter_context(tc.tile_pool(name="ps1", bufs=1, space="PSUM"))

        # ---------------- setup: weights to SBUF
        def ld(name, shape, dtype=f32, src=None, tag=None):
            t = wp.tile(shape, dtype, tag=tag or name)
            ap = D[name][:] if src is None else src
            if dtype == f32r:
                ap = ap.bitcast(f32r)
            nc.sync.dma_start(out=t[:], in_=ap)
            return t

        W_gc = ld("W_gc", [IN_DIM, HID], f32r)
        b_gc = ld("b_gc", [HID, 1])
        W_ri = ld("W_ro_in", [HID, HID], f32r); b_ri = ld("b_ro_in", [HID, 1])
        W_ro = ld("W_ro_out", [HID, HID], f32r); b_ro = ld("b_ro_out", [HID, 1])
        Wc1 = ld("Wc1", [HID, HID], f32r); bc1 = ld("bc1", [HID, 1])
        Wc2 = ld("Wc2", [HID, HID], f32r); bc2 = ld("bc2", [HID, 1])
        Wf1 = ld("Wf1_r", [HID, 2, 2 * HID],
                 src=D["Wf1_r"][:].rearrange("k h m -> h k m"))
        bf1 = ld("bf1_r", [HID, 2, 1],
                 src=D["bf1_r"][:].rearrange("k h o -> h k o"))
        Wf2 = ld("Wf2_r", [HID, 2, 1],
                 src=D["Wf2_r"][:].rearrange("k h o -> h k o"))
        bf2 = ld("bf2", [1, 1])
        embT = ld("embedT", [HID, VOCAB], f32r)
        KT = [ld("K%dT" % (l + 1), [CHANNELS[l], 3, CHANNELS[l + 1]], f32r)
              for l in range(4)]
        cb = [ld("cb%d" % (l + 1), [CHANNELS[l + 1], 1]) for l in range(4)]
        Sg = ld("S", [P, NT, GPC], f32r,
                src=D["S"][:].rearrange("t p g -> p t g"))
        ixs = [ld("ix%d" % T, [P, tok_total[T] // 16], i16) for T in range(3)]
        dginp = ld("deg_in_perm", [P, NT])
        dts = [ld("dtok%d" % T, [P, tok_total[T] // P]) for T in range(3)]

        xb = []
        for l in range(3):
            pair = []
            for j in range(2):
                t = wp.tile([CHANNELS[l + 1], LCONV], f32r,
                            tag="xb%d_%d" % (l, j))
                nc.vector.memset(t[:, 0:1].bitcast(dt.float32), 0.0)
                nc.vector.memset(t[:, LCONV - 1:LCONV].bitcast(dt.float32),
                                 0.0)
                pair.append(t)
            xb.append(pair)

        ident = wp.tile([P, P], f32, tag="ident")
        make_identity(nc, ident[:])
        identr = wp.tile([P, P], f32r, tag="identr")
        nc.vector.tensor_copy(identr[:], ident[:])

        # rsqrt factors: w = sqrt(1/deg) per gather token / per dst lane
        for T in range(3):
            nc.vector.reciprocal(dts[T][:], dts[T][:])
            nc.scalar.sqrt(dts[T][:], dts[T][:])
        rdgi = wp.tile([P, NT], f32, tag="rdgi")
        nc.vector.reciprocal(rdgi[:], dginp[:])
        nc.scalar.sqrt(rdgi[:], rdgi[:])

        # M1rep[32s:32s+25, t, :] = embed @ K1_t^T replicated at 4 offsets
        M1rep = wp.tile([P, 3, CHANNELS[1]], f32r, tag="m1rep")
        for t in range(3):
            pm = ps1.tile([VOCAB, CHANNELS[1]], f32, space="PSUM", tag="ps1a")
            nc.tensor.matmul(pm[:], embT[:], KT[0][:, t, :], start=True,
                             stop=True)
            nc.scalar.copy(M1rep[:VOCAB, t, :], pm[:])
        for srow in range(1, 4):
            nc.sync.dma_start(out=M1rep[32 * srow:32 * srow + VOCAB, :, :],
                              in_=M1rep[:VOCAB, :, :])

        # ---------------- interleaved: conv proteins + gather groups
        acc = {}

        def emit_group(grp, after_protein=None):
            ohg = cvp.tile([P, LCONV], f32r, tag="ohg")
            nc.sync.dma_start(out=ohg[:], in_=D["onehot"][grp].bitcast(f32r))
            for srow in range(4):
                p = grp * 4 + srow
                b0 = 32 * srow
                xs = None
                for l in range(4):
                    cin, cout = CHANNELS[l], CHANNELS[l + 1]
                    for cchunk in range(2):
                        c0 = cchunk * 500
                        pps = pcv.tile([cout, 500], f32, space="PSUM",
                                       tag="cps")
                        for tap in range(3):
                            if l == 0:
                                lhsT = M1rep[b0:b0 + VOCAB, tap, :]
                                rhs = ohg[b0:b0 + VOCAB,
                                          c0 + tap:c0 + tap + 500]
                                tpos = (96, 0) if srow == 3 else None
                            else:
                                lhsT = KT[l][:, tap, :]
                                rhs = xs[:cin, c0 + tap:c0 + tap + 500]
                                tpos = None
                            nc.tensor.matmul(pps[:], lhsT, rhs,
                                             start=(tap == 0), stop=(tap == 2),
                                             tile_position=tpos)
                        if l < 3:
                            nc.scalar.activation(
                                xb[l][p % 2][:, 1 + c0:1 + c0 + 500],
                                pps[:], AF.Relu, bias=cb[l][:])
                        else:
                            nc.vector.reduce_max(
                                out=chunkmax[:, cchunk, p:p + 1],
                                in_=pps[:, :500], axis=AX.X)
                    if l < 3:
                        xs = xb[l][p % 2]
                if after_protein is not None:
                    after_protein(p)

        gjobs = []
        for T in range(3):
            off = 0
            for ins in sched[T]:
                gjobs.append((T, off, ins))
                off += sum(k * P for (_, k) in ins)

        def emit_gather(job, qn):
            T, off, ins = job
            ntok = sum(k * P for (_, k) in ins)
            g = gp.tile([P, ntok // P, P], f32, tag="g%d" % (qn % 6))
            nc.gpsimd.dma_gather(
                out_ap=g[:], in_ap=tabs[T][:],
                idxs_ap=ixs[T][:, off // 16:(off + ntok) // 16],
                num_idxs=ntok, num_idxs_reg=ntok, elem_size=P,
                single_packet=False, queue_num=qn % 4)
            blk0 = off // P
            nc.vector.tensor_tensor(
                out=g[:, :, :IN_DIM],
                in0=g[:, :, :IN_DIM],
                in1=dts[T][:, blk0:blk0 + ntok // P, None]
                    .to_broadcast([P, ntok // P, IN_DIM]),
                op=ALU.mult)
            boff = 0
            for (t, k) in ins:
                view = g[:, boff:boff + k, :IN_DIM].rearrange("p k d -> p d k")
                if t not in acc:
                    a = accp.tile([P, IN_DIM], f32, tag="acc%d" % t)
                    acc[t] = a
                    nc.vector.tensor_reduce(out=a[:], in_=view, axis=AX.X,
                                            op=ALU.add)
                else:
                    tmp = gp.tile([P, IN_DIM], f32, tag="rtmp")
                    nc.vector.tensor_reduce(out=tmp[:], in_=view, axis=AX.X,
                                            op=ALU.add)
                    nc.vector.tensor_add(out=acc[t][:], in0=acc[t][:],
                                         in1=tmp[:])
                boff += k

        pmax = wp.tile([P, PPC], f32, tag="pmax")
        chunkmax = wp.tile([P, 2, PPC], f32, tag="chunkmax")
        gq = list(gjobs)
        qst = [0]

        def drain(p):
            while gq and len(gq) > (PPC - 1 - p) * len(gjobs) // PPC:
                emit_gather(gq.pop(0), qst[0])
                qst[0] += 1

        for grp in range(PPC // 4):
            emit_group(grp, after_protein=drain)
        qn = qst[0]
        while gq:
            emit_gather(gq.pop(0), qn)
            qn += 1
        # pmax = relu(max(chunk maxes) + cb4)
        mxt = wp.tile([P, PPC], f32, tag="mxt")
        nc.vector.tensor_reduce(out=mxt[:],
                                in_=chunkmax[:].rearrange("p c q -> p q c"),
                                axis=AX.X, op=ALU.max)
        nc.scalar.activation(pmax[:], mxt[:], AF.Relu, bias=cb[3][:])
        # scale by rsqrt(deg_in)
        for t in range(NT):
            nc.vector.tensor_scalar_mul(acc[t][:], acc[t][:],
                                        rdgi[:, t:t + 1])

        # ---------------- GNN matmul chain (fp32)
        hg_ps = ps1.tile([GPC, HID], f32, space="PSUM", tag="hgps")
        for t in range(NT):
            tp = pgn.tile([IN_DIM, P], f32, space="PSUM", tag="gps")
            nc.tensor.transpose(tp[:], acc[t][:], ident[:])
            aggT = gnp.tile([IN_DIM, P], f32r, tag="aggT")
            nc.scalar.copy(aggT[:], tp[:])
            hps = pgn.tile([HID, P], f32, space="PSUM", tag="gps")
            nc.tensor.matmul(hps[:], W_gc[:], aggT[:], start=True, stop=True)
            h = gnp.tile([HID, P], f32r, tag="h")
            nc.scalar.activation(h[:], hps[:], AF.Relu, bias=b_gc[:])
            x1ps = pgn.tile([HID, P], f32, space="PSUM", tag="gps")
            nc.tensor.matmul(x1ps[:], W_ri[:], h[:], start=True, stop=True)
            x1 = gnp.tile([HID, P], f32r, tag="x1")
            nc.scalar.activation(x1[:], x1ps[:], AF.Identity, bias=b_ri[:])
            x2ps = pgn.tile([HID, P], f32, space="PSUM", tag="gps")
            nc.tensor.matmul(x2ps[:], W_ro[:], x1[:], start=True, stop=True)
            x2 = gnp.tile([HID, P], f32r, tag="x2")
            nc.scalar.activation(x2[:], x2ps[:], AF.Identity, bias=b_ro[:])
            x2t = pgn.tile([P, HID], f32r, space="PSUM", tag="gps")
            nc.tensor.transpose(x2t[:], x2[:], identr[:])
            x2n = gnp.tile([P, HID], f32r, tag="x2n")
            nc.scalar.copy(x2n[:], x2t[:])
            nc.tensor.matmul(hg_ps[:], Sg[:, t, :], x2n[:],
                             start=(t == 0), stop=(t == NT - 1),
                             skip_group_check=True)
        hgT = wp.tile([GPC, HID], f32, tag="hgT")
        nc.scalar.activation(hgT[:], hg_ps[:], AF.Relu)
        hgt_ps = pgn.tile([HID, GPC], f32, space="PSUM", tag="gps")
        nc.tensor.transpose(hgt_ps[:], hgT[:], ident[:GPC, :GPC])
        hg = wp.tile([HID, GPC], f32r, tag="hg")
        nc.scalar.copy(hg[:], hgt_ps[:])
        # compound FC
        c1ps = pgn.tile([HID, GPC], f32, space="PSUM", tag="gps")
        nc.tensor.matmul(c1ps[:], Wc1[:], hg[:], start=True, stop=True)
        cv1 = wp.tile([HID, GPC], f32r, tag="cv1")
        nc.scalar.activation(cv1[:], c1ps[:], AF.Relu, bias=bc1[:])
        c2ps = pgn.tile([HID, GPC], f32, space="PSUM", tag="gps")
        nc.tensor.matmul(c2ps[:], Wc2[:], cv1[:], start=True, stop=True)
        cv2 = wp.tile([HID, GPC], f32, tag="cv2")
        nc.scalar.activation(cv2[:], c2ps[:], AF.Relu, bias=bc2[:])
        # head: z = [cv2; pmax]
        zin = [cv2, pmax]
        z2 = []
        for mc in range(2):
            zps = pgn.tile([HID, GPC], f32, space="PSUM", tag="gps")
            for kc in range(2):
                nc.tensor.matmul(zps[:], Wf1[:, kc, mc * HID:(mc + 1) * HID],
                                 zin[kc][:, :GPC], start=(kc == 0),
                                 stop=(kc == 1))
            zt = wp.tile([HID, GPC], f32, tag="z2_%d" % mc)
            nc.scalar.activation(zt[:], zps[:], AF.Relu, bias=bf1[:, mc, :])
            z2.append(zt)
        ops = ps1.tile([1, GPC], f32, space="PSUM", tag="ps1a")
        for kc in range(2):
            nc.tensor.matmul(ops[:], Wf2[:, kc, :], z2[kc][:],
                             start=(kc == 0), stop=(kc == 1))
        ot = wp.tile([1, GPC], f32, tag="ot")
        nc.scalar.activation(ot[:], ops[:], AF.Sigmoid, bias=bf2[:1, :])
        nc.sync.dma_start(out=out_d[:], in_=ot[:])

    nc.compile()
    return nc


def kernel(**inputs):
    shared, percore, meta = _host_prep(inputs)
    nc = _build(shared, meta)
    in_maps = []
    for c in range(NCORES):
        m = dict(shared)
        m.update(percore[c])
        in_maps.append(m)
    res = run_bass_kernel_spmd(nc, in_maps, list(range(NCORES)))
    out = np.concatenate([res.results[c]["out"].reshape(GPC)
                          for c in range(NCORES)])
    return out.reshape(B, 1).astype(np.float32)


if __name__ == "__main__":
    sys.path.insert(0, "/root/problem")
    import jax
    import reference
    with jax.default_device(jax.devices("cpu")[0]):
        inputs = {k: np.asarray(v) for k, v in reference.setup_inputs().items()}
        exp = np.asarray(reference.reference(**inputs))
    got = kernel(**inputs)
    err = np.abs(got - exp).max()
    rel = err / max(np.abs(exp).max(), 1e-9)
    print("max abs err:", err, " rel:", rel)



# revision 19
# speedup vs baseline: 1.6855x; 1.6855x over previous
"""CPI_DGLLife kernel for 8 Trainium2 NeuronCores (SPMD).

GCN over a 65536-node graph + protein conv1d branch + CPI head.
Sharding: data-parallel over the 512-graph batch (64 graphs / core).

v2 design:
- Phase-separated schedule: all edge gathers issue up-front on the gpsimd
  SWDGE queues (transpose-mode, bf16, 256B tokens); the protein conv runs
  dense on tensor/scalar/vector with no cross-engine blocking; reductions
  + GNN matmul chain run at the end.
- bf16 matmuls everywhere (1 cyc/row at any free-dim size).
- rsqrt(deg_out) folded into the gather tables; the two readout linears
  folded into one matmul (no activation between them); conv biases folded
  into the matmuls via a ones-row in the rhs (layers 1, 2, 4).
- 2-table split (32767/32767/2) + per-core lexicographic degree bundling
  cuts gather token padding from 2.0x to ~1.2x.
"""
import sys
sys.path.insert(0, "/opt/trn_rl_repo")
import contextlib
import numpy as np
import ml_dtypes

import concourse.bass as bass
import concourse.bacc as bacc
import concourse.tile as tile
from concourse import mybir
from concourse.bass_utils import run_bass_kernel_spmd
from concourse.masks import make_identity

bf16 = ml_dtypes.bfloat16
dt = mybir.dt
AF = mybir.ActivationFunctionType
ALU = mybir.AluOpType
AX = mybir.AxisListType

P = 128
N, E, B, L = 65536, 262144, 512, 1000
IN_DIM, HID, VOCAB = 74, 128, 25
CHANNELS = [HID, 96, 128, IN_DIM, HID]
NCORES = 8
GPC = B // NCORES              # graphs per core = 64
PPC = GPC                      # proteins per core = 64
TBASES = [0, 32767, 65534]
TNN = [32767, 32767, 2]
TOKCAP = 4096                  # max tokens per dma_gather instruction
NQ = 4


# ------------------------------------------------------------------ host prep
def _host_prep(inputs):
    graph_ids = np.asarray(inputs["graph_ids"])
    src = np.concatenate([np.asarray(inputs["edge_src"]).astype(np.int64),
                          np.arange(N, dtype=np.int64)])
    dst = np.concatenate([np.asarray(inputs["edge_dst"]).astype(np.int64),
                          np.arange(N, dtype=np.int64)])
    deg_out = np.bincount(src, minlength=N).astype(np.float32)
    deg_in = np.bincount(dst, minlength=N).astype(np.float32)
    NE = len(src)

    # gather tables: bf16 [rows, 128], row v+1 = X[v] * rsqrt(deg_out[v])
    nf = np.asarray(inputs["node_feats"], np.float32)
    nfs = nf * (1.0 / np.sqrt(deg_out))[:, None]
    tabs = []
    for T in range(3):
        tb = np.zeros((TNN[T] + 1, P), np.float32)
        tb[1:1 + TNN[T], :IN_DIM] = nfs[TBASES[T]:TBASES[T] + TNN[T]]
        tabs.append(tb.astype(bf16))

    tbl_of = np.digitize(src, TBASES[1:])          # table of each edge's src
    loc_of = (src - np.asarray(TBASES)[tbl_of] + 1).astype(np.int64)

    # per-dst-node per-table edge counts
    cnt = np.zeros((N, 3), np.int64)
    np.add.at(cnt, (dst, tbl_of), 1)

    core_node_lo = np.searchsorted(graph_ids, np.arange(0, B + 1, GPC))
    ncore_nodes = core_node_lo[1:] - core_node_lo[:-1]
    NT = int(np.ceil(ncore_nodes.max() / P))
    NPAD = NT * P

    # per-core node permutation: lexicographic descending by (c1, c0)
    perm = np.full((NCORES, NPAD), -1, np.int64)
    for c in range(NCORES):
        lo, hi = int(core_node_lo[c]), int(core_node_lo[c + 1])
        cc = cnt[lo:hi]
        order = np.lexsort((-cc[:, 0], -cc[:, 1])) + lo
        perm[c, :hi - lo] = order

    # k per (tile, table): max over cores and lanes (shared SPMD schedule)
    G = 128
    NG = P // G
    cnt_perm = np.zeros((NCORES, NPAD, 3), np.int64)
    m = perm >= 0
    cnt_perm[m] = cnt[perm[m]]
    kg = cnt_perm.reshape(NCORES, NT, NG, G, 3).max(axis=3).max(axis=0)

    # token stream offsets per (table, tile, group); instruction packing.
    # instructions are disjoint [off, off+ntok) ranges, each %128 tokens
    # (padding tokens point at table row 0 = zeros).
    tok_off = np.full((3, NT, NG), -1, np.int64)
    tok_total = [0, 0, 0]
    sched = []  # (T, off, ntok)
    for T in range(3):
        off = 0
        cur_off = 0
        for t in range(NT):
            blk = int(kg[t, :, T].sum()) * G
            if blk == 0:
                continue
            if off - cur_off + blk > TOKCAP and off > cur_off:
                off = int(np.ceil(off / 128)) * 128
                sched.append((T, cur_off, off - cur_off))
                cur_off = off
            for g in range(NG):
                if kg[t, g, T] > 0:
                    tok_off[T, t, g] = off
                    off += int(kg[t, g, T]) * G
        if off > cur_off:
            off = int(np.ceil(off / 128)) * 128
            sched.append((T, cur_off, off - cur_off))
        tok_total[T] = max(off, 128)

    # node -> (core, padded position)
    pos_of = np.full(N, -1, np.int64)
    core_of = np.full(N, -1, np.int64)
    for c in range(NCORES):
        pm = perm[c]
        v = pm >= 0
        pos_of[pm[v]] = np.arange(NPAD)[v]
        core_of[pm[v]] = c

    # slot of each edge within its (core, tile, lane, table) group
    ec = core_of[dst]
    et = pos_of[dst] // P
    ep = pos_of[dst] % P
    key = (((ec * NT + et) * P + ep) * 3 + tbl_of)
    order = np.argsort(key, kind="stable")
    ks = key[order]
    starts = np.r_[0, np.flatnonzero(np.diff(ks)) + 1]
    grp_len = np.diff(np.r_[starts, NE])
    slot_sorted = np.arange(NE) - np.repeat(starts, grp_len)
    slot = np.empty(NE, np.int64)
    slot[order] = slot_sorted

    # token position (non-transpose layout): off(tile,T) + slot*128 + lane
    tok_pos = tok_off[tbl_of, et, 0] + slot * P + ep
    idx_flat = [np.zeros((NCORES, tok_total[T]), np.int16) for T in range(3)]
    for T in range(3):
        mT = tbl_of == T
        idx_flat[T][ec[mT], tok_pos[mT]] = loc_of[mT].astype(np.int16)

    def wrap(a):  # token-major -> wrapped [128, tokens//16]
        ncol = a.shape[1] // 16
        w = a.reshape(a.shape[0], ncol, 16).transpose(0, 2, 1)
        return np.ascontiguousarray(np.tile(w, (1, 8, 1)))

    idx_wrapped = [wrap(ix) for ix in idx_flat]

    # rsqrt(deg_in) per permuted lane, laid out [P, NT]
    rdgi = np.ones((NCORES, NPAD), np.float32)
    rdgi[m] = 1.0 / np.sqrt(deg_in[perm[m]])
    rdgi_pt = np.ascontiguousarray(
        rdgi.reshape(NCORES, NT, P).transpose(0, 2, 1))

    # S tiles: [NT, P, GPC] graph membership (bf16), node-major partitions
    S = np.zeros((NCORES, NT, P, GPC), np.float32)
    cnt_g = np.zeros((NCORES, GPC), np.float32)
    for c in range(NCORES):
        pm = perm[c]
        valid = pm >= 0
        g = graph_ids[pm[valid]] - c * GPC
        tt = np.arange(NPAD)[valid] // P
        pp = np.arange(NPAD)[valid] % P
        S[c, tt, pp, g] = 1.0
        np.add.at(cnt_g[c], g, 1.0)
    Sb = S.astype(bf16)

    # reduce plan per tile: (table, k, token offset) for each live table
    tile_tabs = []
    for t in range(NT):
        entry = [(T, int(kg[t, 0, T]), int(tok_off[T, t, 0]))
                 for T in range(3) if kg[t, 0, T] > 0]
        tile_tabs.append(entry)
    live = [len(tile_tabs[t]) > 0 for t in range(NT)]

    # protein one-hot, tap-stacked at 32-aligned rows + ones row at 96
    seq = np.asarray(inputs["protein_seq"]).reshape(NCORES, PPC, L)
    ohS = np.zeros((NCORES, PPC, 97, L), bf16)
    iot = np.arange(VOCAB)[None, None, :, None]
    one = np.float32(1)
    ohS[:, :, 0:VOCAB, 1:] = (seq[:, :, None, :-1] == iot) * one
    ohS[:, :, 32:32 + VOCAB, :] = (seq[:, :, None, :] == iot) * one
    ohS[:, :, 64:64 + VOCAB, :-1] = (seq[:, :, None, 1:] == iot) * one
    ohS[:, :, 96, :] = one

    # weights
    f32 = np.float32

    def b16(x):
        return np.ascontiguousarray(np.asarray(x, np.float32).astype(bf16))

    W_ri = np.asarray(inputs["W_ro_in"], f32)
    W_ro = np.asarray(inputs["W_ro_out"], f32)
    b_ri = np.asarray(inputs["b_ro_in"], f32)
    b_ro = np.asarray(inputs["b_ro_out"], f32)
    W_r2 = W_ri @ W_ro
    b_r2 = b_ri @ W_ro + b_ro                     # [HID]
    B2 = b_r2[None, :, None] * cnt_g[:, None, :]  # [NCORES, HID, GPC]

    # conv weights, tap-sliced lhsT with bias rows
    K1 = np.asarray(inputs["K1"], f32)            # [96, 128, 3]
    K2 = np.asarray(inputs["K2"], f32)            # [128, 96, 3]
    K3 = np.asarray(inputs["K3"], f32)            # [74, 128, 3]
    K4 = np.asarray(inputs["K4"], f32)            # [128, 74, 3]
    KT2e = np.zeros((97, 3, 128), f32)
    KT2e[:96] = K2.transpose(1, 2, 0)
    KT2e[96, 0, :] = np.asarray(inputs["cb2"], f32)
    KT3 = K3.transpose(1, 2, 0).copy()            # [128, 3, 74]
    KT4e = np.zeros((75, 3, 128), f32)
    KT4e[:74] = K4.transpose(1, 2, 0)
    KT4e[74, 0, :] = np.asarray(inputs["cb4"], f32)

    shared = {
        "tab0": tabs[0], "tab1": tabs[1], "tab2": tabs[2],
        "embT": b16(np.asarray(inputs["embed"], f32).T),      # [HID, 25]
        "K1T": b16(K1.transpose(1, 2, 0)),                    # [HID, 3, 96]
        "cb1row": b16(np.asarray(inputs["cb1"], f32).reshape(1, 96)),
        "KT2e": b16(KT2e), "KT3": b16(KT3), "KT4e": b16(KT4e),
        "cb3": np.asarray(inputs["cb3"], f32).reshape(IN_DIM, 1),
        "W_gc": b16(np.asarray(inputs["W_gc"], f32)),         # [74, HID]
        "b_gc": np.asarray(inputs["b_gc"], f32).reshape(HID, 1),
        "W_r2": b16(W_r2),
        "Wc1": b16(np.asarray(inputs["Wc1"], f32)),
        "bc1": np.asarray(inputs["bc1"], f32).reshape(HID, 1),
        "Wc2": b16(np.asarray(inputs["Wc2"], f32)),
        "bc2": np.asarray(inputs["bc2"], f32).reshape(HID, 1),
        "Wf1_r": b16(np.asarray(inputs["Wf1"], f32).reshape(2, HID, 2 * HID)
                     .transpose(1, 0, 2)),                    # [HID, 2, 256]
        "bf1_r": np.ascontiguousarray(
            np.asarray(inputs["bf1"], f32).reshape(2, HID, 1)
            .transpose(1, 0, 2)),                             # [HID, 2, 1]
        "Wf2_r": b16(np.asarray(inputs["Wf2"], f32).reshape(2, HID, 1)
                     .transpose(1, 0, 2)),                    # [HID, 2, 1]
        "bf2": np.asarray(inputs["bf2"], f32).reshape(1, 1),
        "ones2": np.ones((1, 1002), bf16),
    }
    percore = []
    for c in range(NCORES):
        percore.append({
            "ix0": idx_wrapped[0][c],
            "ix1": idx_wrapped[1][c],
            "ix2": idx_wrapped[2][c],
            "rdgi": np.ascontiguousarray(rdgi_pt[c]),
            "S": np.ascontiguousarray(Sb[c]),
            "ohS": np.ascontiguousarray(ohS[c]),
            "B2": np.ascontiguousarray(B2[c]),
        })
    meta = dict(NT=NT, sched=sched, tok_total=tok_total,
                tile_tabs=tile_tabs, live=live)
    return shared, percore, meta


# --------------------------------------------------------------- device build
def _build(shared, meta):
    NT = meta["NT"]
    sched = meta["sched"]
    tok_total = meta["tok_total"]
    tile_tabs = meta["tile_tabs"]
    live = meta["live"]

    nc = bacc.Bacc("TRN2", target_bir_lowering=False, debug=False,
                   num_devices=NCORES, num_swdge_queues=NQ)
    f32, bf, i16 = dt.float32, dt.bfloat16, dt.int16

    D = {k: nc.dram_tensor(k, list(v.shape), dt.from_np(v.dtype),
                           kind="ExternalInput")
         for k, v in shared.items()}
    for T in range(3):
        D["ix%d" % T] = nc.dram_tensor("ix%d" % T, [P, tok_total[T] // 16],
                                       i16, kind="ExternalInput")
    D["rdgi"] = nc.dram_tensor("rdgi", [P, NT], f32, kind="ExternalInput")
    D["S"] = nc.dram_tensor("S", [NT, P, GPC], bf, kind="ExternalInput")
    D["ohS"] = nc.dram_tensor("ohS", [PPC, 97, L], bf, kind="ExternalInput")
    D["B2"] = nc.dram_tensor("B2", [HID, GPC], f32, kind="ExternalInput")
    out_d = nc.dram_tensor("out", [1, GPC], f32, kind="ExternalOutput")
    tabs = [D["tab%d" % T] for T in range(3)]

    with tile.TileContext(nc) as tc, contextlib.ExitStack() as ctx:
        wp = ctx.enter_context(tc.tile_pool(name="wp", bufs=1))
        gpool = ctx.enter_context(tc.tile_pool(name="gpool", bufs=1))
        ohp = ctx.enter_context(tc.tile_pool(name="ohp", bufs=2))
        redp = ctx.enter_context(tc.tile_pool(name="redp", bufs=4))
        aggp = ctx.enter_context(tc.tile_pool(name="aggp", bufs=1))
        gnp = ctx.enter_context(tc.tile_pool(name="gnp", bufs=3))
        pcv = ctx.enter_context(tc.tile_pool(name="pcv", bufs=4, space="PSUM"))
        pgn = ctx.enter_context(tc.tile_pool(name="pgn", bufs=2, space="PSUM"))
        ps1 = ctx.enter_context(tc.tile_pool(name="ps1", bufs=1, space="PSUM"))

        # ---------------- setup: weights/indices to SBUF
        def ld(name, shape, dtype, src=None, tag=None):
            t = wp.tile(shape, dtype, tag=tag or name)
            nc.sync.dma_start(out=t[:], in_=D[name][:] if src is None else src)
            return t

        ixs = [ld("ix%d" % T, [P, tok_total[T] // 16], i16) for T in range(3)]
        embT = ld("embT", [HID, VOCAB], bf)
        K1T = ld("K1T", [HID, 3, 96], bf)
        KT2e = ld("KT2e", [97, 3, 128], bf)
        KT3 = ld("KT3", [128, 3, IN_DIM], bf)
        KT4e = ld("KT4e", [75, 3, 128], bf)
        cb3 = ld("cb3", [IN_DIM, 1], f32)
        W_gc = ld("W_gc", [IN_DIM, HID], bf)
        b_gc = ld("b_gc", [HID, 1], f32)
        W_r2 = ld("W_r2", [HID, HID], bf)
        Wc1 = ld("Wc1", [HID, HID], bf); bc1 = ld("bc1", [HID, 1], f32)
        Wc2 = ld("Wc2", [HID, HID], bf); bc2 = ld("bc2", [HID, 1], f32)
        Wf1 = ld("Wf1_r", [HID, 2, 2 * HID], bf)
        bf1 = ld("bf1_r", [HID, 2, 1], f32)
        Wf2 = ld("Wf2_r", [HID, 2, 1], bf)
        bf2 = ld("bf2", [1, 1], f32)
        rdgi = ld("rdgi", [P, NT], f32)
        Sg = ld("S", [P, NT, GPC], bf,
                src=D["S"][:].rearrange("t p g -> p t g"))
        B2 = ld("B2", [HID, GPC], f32)

        # M1stack [97, 96]: rows 32t+v = embed[v] @ K1[:,:,t]^T; row 96 = cb1
        # (tap blocks at 32-aligned partition bases; gap rows zeroed so the
        # matmul against zero one-hot rows cannot pick up NaN garbage)
        M1 = wp.tile([97, 96], bf, tag="m1")
        nc.vector.memset(M1[:], 0.0)
        nc.sync.dma_start(out=M1[96:97, :], in_=D["cb1row"][:])
        for t in range(3):
            pm = ps1.tile([VOCAB, 96], f32, space="PSUM", tag="ps1a")
            nc.tensor.matmul(pm[:], embT[:], K1T[:, t, :], start=True,
                             stop=True)
            nc.scalar.copy(M1[32 * t:32 * t + VOCAB, :], pm[:])

        # xs conv buffers: ones rows (matmul-folded bias) + zero guard cols
        x1 = wp.tile([97, 1002], bf, tag="xs1")
        x2 = wp.tile([128, 1002], bf, tag="xs2")
        x3 = wp.tile([75, 1002], bf, tag="xs3")
        nc.sync.dma_start(out=x1[96:97, :], in_=D["ones2"][:])
        nc.sync.dma_start(out=x3[74:75, :], in_=D["ones2"][:])
        for tl, nr in ((x1, 96), (x2, 128), (x3, 74)):
            nc.vector.memset(tl[0:nr, 0:1], 0.0)
            nc.vector.memset(tl[0:nr, 1001:1002], 0.0)

        chunkmax = wp.tile([HID, 2, PPC], f32, tag="chunkmax")
        ident = wp.tile([P, P], f32, tag="ident")
        make_identity(nc, ident[:])

        # ---------------- phase 1a: issue all gathers (gpsimd queues)
        gts = []
        for i, (T, off, ntok) in enumerate(sched):
            g = gpool.tile([P, ntok // P, P], bf, tag="g%d" % i)
            nc.gpsimd.dma_gather(
                out_ap=g[:], in_ap=tabs[T][:],
                idxs_ap=ixs[T][:, off // 16:(off + ntok) // 16],
                num_idxs=ntok, num_idxs_reg=ntok, elem_size=P,
                single_packet=False, queue_num=i % NQ)
            gts.append((T, off, ntok, g))

        def g_slice(T, toff, width):
            # locate the gather tile holding table-T tokens [toff, toff+width)
            for (Tg, off, ntok, g) in gts:
                if Tg == T and off <= toff and toff + width <= off + ntok:
                    b = (toff - off) // P
                    return g[:, b:b + width // P, 0:IN_DIM]
            raise AssertionError("token range not found")

        # ---------------- phase 1b: protein conv stack (dense)
        for p in range(PPC):
            oh = ohp.tile([97, L], bf, tag="oh%d" % (p % 2))
            nc.sync.dma_start(out=oh[:], in_=D["ohS"][p])
            # conv1: single matmul per chunk (taps stacked, bias row)
            for ci, c0 in enumerate((0, 500)):
                ps = pcv.tile([96, 500], f32, space="PSUM", tag="cps")
                nc.tensor.matmul(ps[:], M1[:], oh[:, c0:c0 + 500],
                                 start=True, stop=True)
                nc.vector.tensor_scalar(
                    out=x1[0:96, 1 + c0:501 + c0], in0=ps[:],
                    scalar1=0.0, scalar2=None, op0=ALU.max)
            # conv2/3/4: 3 taps, 2 chunks, taps outer (weight reuse)
            for lyr, (KT, xin, nin) in enumerate(
                    ((KT2e, x1, 97), (KT3, x2, 128), (KT4e, x3, 75))):
                pss = [pcv.tile([CHANNELS[lyr + 2], 500], f32, space="PSUM",
                                tag="cps", name="cps%d" % ci)
                       for ci in range(2)]
                for tap in range(3):
                    for ci, c0 in enumerate((0, 500)):
                        nc.tensor.matmul(
                            pss[ci][:], KT[:, tap, :],
                            xin[0:nin, c0 + tap:c0 + tap + 500],
                            start=(tap == 0), stop=(tap == 2))
                for ci, c0 in enumerate((0, 500)):
                    if lyr == 0:    # -> xs2, relu via DVE (bias in psum)
                        nc.vector.tensor_scalar(
                            out=x2[:, 1 + c0:501 + c0], in0=pss[ci][:],
                            scalar1=0.0, scalar2=None, op0=ALU.max)
                    elif lyr == 1:  # -> xs3, relu+bias via ACT
                        nc.scalar.activation(
                            x3[0:IN_DIM, 1 + c0:501 + c0], pss[ci][:],
                            AF.Relu, bias=cb3[:])
                    else:           # conv4: max-pool the chunk
                        nc.vector.tensor_reduce(
                            out=chunkmax[:, ci, p:p + 1], in_=pss[ci][:],
                            axis=AX.X, op=ALU.max)

        # ---------------- phase 2: pmax, edge reduces, GNN chain
        mxt = wp.tile([HID, PPC], f32, tag="mxt")
        nc.vector.tensor_reduce(out=mxt[:],
                                in_=chunkmax[:].rearrange("p c q -> p q c"),
                                axis=AX.X, op=ALU.max)
        pmax = wp.tile([HID, PPC], bf, tag="pmax")
        nc.scalar.activation(pmax[:], mxt[:], AF.Relu)

        # per-tile segment sums (node-major), scale, PE-transpose to
        # feature-major bf16 aggT tiles
        aggs = []
        for t in range(NT):
            agg = aggp.tile([IN_DIM, P], bf, tag="agg%d" % t)
            aggs.append(agg)
            if not live[t]:
                nc.vector.memset(agg[:], 0.0)
                continue
            parts = []
            for (T, k, toff) in tile_tabs[t]:
                r = redp.tile([P, IN_DIM], f32,
                              tag="red%d" % (len(parts)), name="red")
                view = g_slice(T, toff, k * P).rearrange("p k d -> p d k")
                nc.vector.tensor_reduce(out=r[:], in_=view, axis=AX.X,
                                        op=ALU.add)
                parts.append(r)
            acc = parts[0]
            for r in parts[1:]:
                nc.vector.tensor_tensor(out=acc[:], in0=acc[:], in1=r[:],
                                        op=ALU.add)
            nc.vector.tensor_scalar_mul(acc[:], acc[:], rdgi[:, t:t + 1])
            tp = pgn.tile([IN_DIM, P], f32, space="PSUM", tag="gps")
            nc.tensor.transpose(tp[:], acc[:], ident[:])
            nc.scalar.copy(agg[:], tp[:])

        # GNN chain per tile, accumulating hgT = sum_t x2n_t^T-free S product
        hgps = ps1.tile([HID, GPC], f32, space="PSUM", tag="hgps")
        lt = [t for t in range(NT)]
        for t in lt:
            hps = pgn.tile([HID, P], f32, space="PSUM", tag="gps")
            nc.tensor.matmul(hps[:], W_gc[:], aggs[t][:], start=True,
                             stop=True)
            h = gnp.tile([HID, P], bf, tag="h")
            nc.scalar.activation(h[:], hps[:], AF.Relu, bias=b_gc[:])
            x2ps = pgn.tile([P, HID], f32, space="PSUM", tag="gps")
            nc.tensor.matmul(x2ps[:], h[:], W_r2[:], start=True, stop=True)
            x2n = gnp.tile([P, HID], bf, tag="x2n")
            nc.scalar.copy(x2n[:], x2ps[:])
            nc.tensor.matmul(hgps[:], x2n[:], Sg[:, t, :],
                             start=(t == lt[0]), stop=(t == lt[-1]),
                             skip_group_check=True)
        hgf = wp.tile([HID, GPC], f32, tag="hgf")
        nc.vector.scalar_tensor_tensor(out=hgf[:], in0=hgps[:], scalar=1.0,
                                       in1=B2[:], op0=ALU.mult, op1=ALU.add)
        hg = wp.tile([HID, GPC], bf, tag="hg")
        nc.scalar.activation(hg[:], hgf[:], AF.Relu)
        # compound FC
        c1ps = pgn.tile([HID, GPC], f32, space="PSUM", tag="gps")
        nc.tensor.matmul(c1ps[:], Wc1[:], hg[:], start=True, stop=True)
        cv1 = wp.tile([HID, GPC], bf, tag="cv1")
        nc.scalar.activation(cv1[:], c1ps[:], AF.Relu, bias=bc1[:])
        c2ps = pgn.tile([HID, GPC], f32, space="PSUM", tag="gps")
        nc.tensor.matmul(c2ps[:], Wc2[:], cv1[:], start=True, stop=True)
        cv2 = wp.tile([HID, GPC], bf, tag="cv2")
        nc.scalar.activation(cv2[:], c2ps[:], AF.Relu, bias=bc2[:])
        # CPI head: z = [cv2; pmax]
        zin = [cv2, pmax]
        z2 = []
        for mc in range(2):
            zps = pgn.tile([HID, GPC], f32, space="PSUM", tag="gps")
            for kc in range(2):
                nc.tensor.matmul(zps[:], Wf1[:, kc, mc * HID:(mc + 1) * HID],
                                 zin[kc][:, :GPC], start=(kc == 0),
                                 stop=(kc == 1))
            zt = wp.tile([HID, GPC], bf, tag="z2_%d" % mc)
            nc.scalar.activation(zt[:], zps[:], AF.Relu, bias=bf1[:, mc, :])
            z2.append(zt)
        ops = ps1.tile([1, GPC], f32, space="PSUM", tag="ps1a")
        for kc in range(2):
            nc.tensor.matmul(ops[:], Wf2[:, kc, :], z2[kc][:],
                             start=(kc == 0), stop=(kc == 1))
        ot = wp.tile([1, GPC], f32, tag="ot")
        nc.scalar.activation(ot[:], ops[:], AF.Sigmoid, bias=bf2[:1, :])
        nc.sync.dma_start(out=out_d[:], in_=ot[:])

    nc.compile()
    return nc


def kernel(**inputs):
    shared, percore, meta = _host_prep(inputs)
    nc = _build(shared, meta)
    in_maps = []
    for c in range(NCORES):
        m = dict(shared)
        m.update(percore[c])
        in_maps.append(m)
    res = run_bass_kernel_spmd(nc, in_maps, list(range(NCORES)))
    out = np.concatenate([res.results[c]["out"].reshape(GPC)
                          for c in range(NCORES)])
    return out.reshape(B, 1).astype(np.float32)


if __name__ == "__main__":
    sys.path.insert(0, "/root/problem")
    import jax
    import reference
    with jax.default_device(jax.devices("cpu")[0]):
        inputs = {k: np.asarray(v) for k, v in reference.setup_inputs().items()}
        exp = np.asarray(reference.reference(**inputs))
    got = kernel(**inputs)
    err = np.abs(got - exp).max()
    rel = err / max(np.abs(exp).max(), 1e-9)
    print("max abs err:", err, " rel:", rel)


# revision 20
# speedup vs baseline: 2.0033x; 1.1885x over previous
"""CPI_DGLLife kernel for 8 Trainium2 NeuronCores (SPMD).

GCN over a 65536-node graph + protein conv1d branch + CPI head.
Sharding: data-parallel over the 512-graph batch (64 graphs / core).

v2 design:
- Phase-separated schedule: all edge gathers issue up-front on the gpsimd
  SWDGE queues (transpose-mode, bf16, 256B tokens); the protein conv runs
  dense on tensor/scalar/vector with no cross-engine blocking; reductions
  + GNN matmul chain run at the end.
- bf16 matmuls everywhere (1 cyc/row at any free-dim size).
- rsqrt(deg_out) folded into the gather tables; the two readout linears
  folded into one matmul (no activation between them); conv biases folded
  into the matmuls via a ones-row in the rhs (layers 1, 2, 4).
- 2-table split (32767/32767/2) + per-core lexicographic degree bundling
  cuts gather token padding from 2.0x to ~1.2x.
"""
import sys
sys.path.insert(0, "/opt/trn_rl_repo")
import contextlib
import numpy as np
import ml_dtypes

import concourse.bass as bass
import concourse.bacc as bacc
import concourse.tile as tile
from concourse import mybir
from concourse.bass_utils import run_bass_kernel_spmd
from concourse.masks import make_identity

bf16 = ml_dtypes.bfloat16
dt = mybir.dt
AF = mybir.ActivationFunctionType
ALU = mybir.AluOpType
AX = mybir.AxisListType

P = 128
N, E, B, L = 65536, 262144, 512, 1000
IN_DIM, HID, VOCAB = 74, 128, 25
CHANNELS = [HID, 96, 128, IN_DIM, HID]
NCORES = 8
GPC = B // NCORES              # graphs per core = 64
PPC = GPC                      # proteins per core = 64
TBASES = [0, 32767, 65534]
TNN = [32767, 32767, 2]
TOKCAP = 4096                  # max tokens per dma_gather instruction
NQ = 4


# ------------------------------------------------------------------ host prep
def _host_prep(inputs):
    graph_ids = np.asarray(inputs["graph_ids"])
    src = np.concatenate([np.asarray(inputs["edge_src"]).astype(np.int64),
                          np.arange(N, dtype=np.int64)])
    dst = np.concatenate([np.asarray(inputs["edge_dst"]).astype(np.int64),
                          np.arange(N, dtype=np.int64)])
    deg_out = np.bincount(src, minlength=N).astype(np.float32)
    deg_in = np.bincount(dst, minlength=N).astype(np.float32)
    NE = len(src)

    # gather tables: bf16 [rows, 128], row v+1 = X[v] * rsqrt(deg_out[v])
    nf = np.asarray(inputs["node_feats"], np.float32)
    nfs = nf * (1.0 / np.sqrt(deg_out))[:, None]
    tabs = []
    for T in range(3):
        tb = np.zeros((TNN[T] + 1, P), np.float32)
        tb[1:1 + TNN[T], :IN_DIM] = nfs[TBASES[T]:TBASES[T] + TNN[T]]
        tabs.append(tb.astype(bf16))

    tbl_of = np.digitize(src, TBASES[1:])          # table of each edge's src
    loc_of = (src - np.asarray(TBASES)[tbl_of] + 1).astype(np.int64)

    # per-dst-node per-table edge counts
    cnt = np.zeros((N, 3), np.int64)
    np.add.at(cnt, (dst, tbl_of), 1)

    core_node_lo = np.searchsorted(graph_ids, np.arange(0, B + 1, GPC))
    ncore_nodes = core_node_lo[1:] - core_node_lo[:-1]
    NT = int(np.ceil(ncore_nodes.max() / P))
    NPAD = NT * P

    # per-core node permutation: lexicographic descending by (c1, c0)
    perm = np.full((NCORES, NPAD), -1, np.int64)
    for c in range(NCORES):
        lo, hi = int(core_node_lo[c]), int(core_node_lo[c + 1])
        cc = cnt[lo:hi]
        order = np.lexsort((-cc[:, 0], -cc[:, 1])) + lo
        perm[c, :hi - lo] = order

    # k per (tile, table): max over cores and lanes (shared SPMD schedule)
    G = 128
    NG = P // G
    cnt_perm = np.zeros((NCORES, NPAD, 3), np.int64)
    m = perm >= 0
    cnt_perm[m] = cnt[perm[m]]
    kg = cnt_perm.reshape(NCORES, NT, NG, G, 3).max(axis=3).max(axis=0)

    # token stream offsets per (table, tile, group); instruction packing.
    # instructions are disjoint [off, off+ntok) ranges, each %128 tokens
    # (padding tokens point at table row 0 = zeros).
    tok_off = np.full((3, NT, NG), -1, np.int64)
    tok_total = [0, 0, 0]
    sched = []  # (T, off, ntok)
    for T in range(3):
        off = 0
        cur_off = 0
        for t in range(NT):
            blk = int(kg[t, :, T].sum()) * G
            if blk == 0:
                continue
            if off - cur_off + blk > TOKCAP and off > cur_off:
                off = int(np.ceil(off / 128)) * 128
                sched.append((T, cur_off, off - cur_off))
                cur_off = off
            for g in range(NG):
                if kg[t, g, T] > 0:
                    tok_off[T, t, g] = off
                    off += int(kg[t, g, T]) * G
        if off > cur_off:
            off = int(np.ceil(off / 128)) * 128
            sched.append((T, cur_off, off - cur_off))
        tok_total[T] = max(off, 128)

    # node -> (core, padded position)
    pos_of = np.full(N, -1, np.int64)
    core_of = np.full(N, -1, np.int64)
    for c in range(NCORES):
        pm = perm[c]
        v = pm >= 0
        pos_of[pm[v]] = np.arange(NPAD)[v]
        core_of[pm[v]] = c

    # slot of each edge within its (core, tile, lane, table) group
    ec = core_of[dst]
    et = pos_of[dst] // P
    ep = pos_of[dst] % P
    key = (((ec * NT + et) * P + ep) * 3 + tbl_of)
    order = np.argsort(key, kind="stable")
    ks = key[order]
    starts = np.r_[0, np.flatnonzero(np.diff(ks)) + 1]
    grp_len = np.diff(np.r_[starts, NE])
    slot_sorted = np.arange(NE) - np.repeat(starts, grp_len)
    slot = np.empty(NE, np.int64)
    slot[order] = slot_sorted

    # token position (non-transpose layout): off(tile,T) + slot*128 + lane
    tok_pos = tok_off[tbl_of, et, 0] + slot * P + ep
    idx_flat = [np.zeros((NCORES, tok_total[T]), np.int16) for T in range(3)]
    for T in range(3):
        mT = tbl_of == T
        idx_flat[T][ec[mT], tok_pos[mT]] = loc_of[mT].astype(np.int16)

    def wrap(a):  # token-major -> wrapped [128, tokens//16]
        ncol = a.shape[1] // 16
        w = a.reshape(a.shape[0], ncol, 16).transpose(0, 2, 1)
        return np.ascontiguousarray(np.tile(w, (1, 8, 1)))

    idx_wrapped = [wrap(ix) for ix in idx_flat]

    # rsqrt(deg_in) per permuted lane, laid out [P, NT]
    rdgi = np.ones((NCORES, NPAD), np.float32)
    rdgi[m] = 1.0 / np.sqrt(deg_in[perm[m]])
    rdgi_pt = np.ascontiguousarray(
        rdgi.reshape(NCORES, NT, P).transpose(0, 2, 1))

    # S tiles: [P, NT, GPC] graph membership (bf16), node-major partitions
    S = np.zeros((NCORES, NT, P, GPC), np.float32)
    cnt_g = np.zeros((NCORES, GPC), np.float32)
    for c in range(NCORES):
        pm = perm[c]
        valid = pm >= 0
        g = graph_ids[pm[valid]] - c * GPC
        tt = np.arange(NPAD)[valid] // P
        pp = np.arange(NPAD)[valid] % P
        S[c, tt, pp, g] = 1.0
        np.add.at(cnt_g[c], g, 1.0)
    Sb = np.ascontiguousarray(S.transpose(0, 2, 1, 3)).astype(bf16)

    # reduce plan per tile: (table, k, token offset) for each live table
    tile_tabs = []
    for t in range(NT):
        entry = [(T, int(kg[t, 0, T]), int(tok_off[T, t, 0]))
                 for T in range(3) if kg[t, 0, T] > 0]
        tile_tabs.append(entry)
    live = [len(tile_tabs[t]) > 0 for t in range(NT)]

    # protein one-hot, tap-stacked at 32-aligned rows + ones row at 96
    seq = np.asarray(inputs["protein_seq"]).reshape(NCORES, PPC, L)
    ohS = np.zeros((NCORES, PPC, 97, L), bf16)
    iot = np.arange(VOCAB)[None, None, :, None]
    one = np.float32(1)
    ohS[:, :, 0:VOCAB, 1:] = (seq[:, :, None, :-1] == iot) * one
    ohS[:, :, 32:32 + VOCAB, :] = (seq[:, :, None, :] == iot) * one
    ohS[:, :, 64:64 + VOCAB, :-1] = (seq[:, :, None, 1:] == iot) * one
    ohS[:, :, 96, :] = one
    # group 4 proteins per DMA: [PPC//4, 97, 4*L]
    ohS = np.ascontiguousarray(
        ohS.reshape(NCORES, PPC // 4, 4, 97, L).transpose(0, 1, 3, 2, 4)
        .reshape(NCORES, PPC // 4, 97, 4 * L))

    # weights
    f32 = np.float32

    def b16(x):
        return np.ascontiguousarray(np.asarray(x, np.float32).astype(bf16))

    W_ri = np.asarray(inputs["W_ro_in"], f32)
    W_ro = np.asarray(inputs["W_ro_out"], f32)
    b_ri = np.asarray(inputs["b_ro_in"], f32)
    b_ro = np.asarray(inputs["b_ro_out"], f32)
    W_r2 = W_ri @ W_ro
    b_r2 = b_ri @ W_ro + b_ro                     # [HID]
    B2 = b_r2[None, :, None] * cnt_g[:, None, :]  # [NCORES, HID, GPC]

    # conv weights, tap-sliced lhsT with bias rows
    K1 = np.asarray(inputs["K1"], f32)            # [96, 128, 3]
    K2 = np.asarray(inputs["K2"], f32)            # [128, 96, 3]
    K3 = np.asarray(inputs["K3"], f32)            # [74, 128, 3]
    K4 = np.asarray(inputs["K4"], f32)            # [128, 74, 3]
    KT2e = np.zeros((97, 3, 128), f32)
    KT2e[:96] = K2.transpose(1, 2, 0)
    KT2e[96, 0, :] = np.asarray(inputs["cb2"], f32)
    KT3 = K3.transpose(1, 2, 0).copy()            # [128, 3, 74]
    KT4e = np.zeros((75, 3, 128), f32)
    KT4e[:74] = K4.transpose(1, 2, 0)
    KT4e[74, 0, :] = np.asarray(inputs["cb4"], f32)

    shared = {
        "tab0": tabs[0], "tab1": tabs[1], "tab2": tabs[2],
        "embT": b16(np.asarray(inputs["embed"], f32).T),      # [HID, 25]
        "K1T": b16(K1.transpose(1, 2, 0)),                    # [HID, 3, 96]
        "cb1row": b16(np.asarray(inputs["cb1"], f32).reshape(1, 96)),
        "KT2e": b16(KT2e), "KT3": b16(KT3), "KT4e": b16(KT4e),
        "cb3": np.asarray(inputs["cb3"], f32).reshape(IN_DIM, 1),
        "W_gc": b16(np.asarray(inputs["W_gc"], f32)),         # [74, HID]
        "b_gc": np.asarray(inputs["b_gc"], f32).reshape(HID, 1),
        "W_r2": b16(W_r2),
        "Wc1": b16(np.asarray(inputs["Wc1"], f32)),
        "bc1": np.asarray(inputs["bc1"], f32).reshape(HID, 1),
        "Wc2": b16(np.asarray(inputs["Wc2"], f32)),
        "bc2": np.asarray(inputs["bc2"], f32).reshape(HID, 1),
        "Wf1_r": b16(np.asarray(inputs["Wf1"], f32).reshape(2, HID, 2 * HID)
                     .transpose(1, 0, 2)),                    # [HID, 2, 256]
        "bf1_r": np.ascontiguousarray(
            np.asarray(inputs["bf1"], f32).reshape(2, HID, 1)
            .transpose(1, 0, 2)),                             # [HID, 2, 1]
        "Wf2_r": b16(np.asarray(inputs["Wf2"], f32).reshape(2, HID, 1)
                     .transpose(1, 0, 2)),                    # [HID, 2, 1]
        "bf2": np.asarray(inputs["bf2"], f32).reshape(1, 1),
        "ones2": np.ones((1, 1002), bf16),
    }
    percore = []
    for c in range(NCORES):
        percore.append({
            "ix0": idx_wrapped[0][c],
            "ix1": idx_wrapped[1][c],
            "ix2": idx_wrapped[2][c],
            "rdgi": np.ascontiguousarray(rdgi_pt[c]),
            "S": np.ascontiguousarray(Sb[c]),
            "ohS": np.ascontiguousarray(ohS[c]),
            "B2": np.ascontiguousarray(B2[c]),
        })
    meta = dict(NT=NT, sched=sched, tok_total=tok_total,
                tile_tabs=tile_tabs, live=live)
    return shared, percore, meta


# --------------------------------------------------------------- device build
def _build(shared, meta):
    NT = meta["NT"]
    sched = meta["sched"]
    tok_total = meta["tok_total"]
    tile_tabs = meta["tile_tabs"]
    live = meta["live"]

    nc = bacc.Bacc("TRN2", target_bir_lowering=False, debug=False,
                   num_devices=NCORES, num_swdge_queues=NQ)
    f32, bf, i16 = dt.float32, dt.bfloat16, dt.int16

    D = {k: nc.dram_tensor(k, list(v.shape), dt.from_np(v.dtype),
                           kind="ExternalInput")
         for k, v in shared.items()}
    for T in range(3):
        D["ix%d" % T] = nc.dram_tensor("ix%d" % T, [P, tok_total[T] // 16],
                                       i16, kind="ExternalInput")
    D["rdgi"] = nc.dram_tensor("rdgi", [P, NT], f32, kind="ExternalInput")
    D["S"] = nc.dram_tensor("S", [P, NT, GPC], bf, kind="ExternalInput")
    D["ohS"] = nc.dram_tensor("ohS", [PPC // 4, 97, 4 * L], bf,
                              kind="ExternalInput")
    D["B2"] = nc.dram_tensor("B2", [HID, GPC], f32, kind="ExternalInput")
    out_d = nc.dram_tensor("out", [1, GPC], f32, kind="ExternalOutput")
    tabs = [D["tab%d" % T] for T in range(3)]

    with tile.TileContext(nc) as tc, contextlib.ExitStack() as ctx:
        wp = ctx.enter_context(tc.tile_pool(name="wp", bufs=1))
        gpool = ctx.enter_context(tc.tile_pool(name="gpool", bufs=1))
        ohp = ctx.enter_context(tc.tile_pool(name="ohp", bufs=2))
        redp = ctx.enter_context(tc.tile_pool(name="redp", bufs=4))
        aggp = ctx.enter_context(tc.tile_pool(name="aggp", bufs=1))
        gnp = ctx.enter_context(tc.tile_pool(name="gnp", bufs=3))
        pcv = ctx.enter_context(tc.tile_pool(name="pcv", bufs=4, space="PSUM"))
        pgn = ctx.enter_context(tc.tile_pool(name="pgn", bufs=2, space="PSUM"))
        ps1 = ctx.enter_context(tc.tile_pool(name="ps1", bufs=1, space="PSUM"))

        # ---------------- setup: weights/indices to SBUF
        def ld(name, shape, dtype, src=None, tag=None):
            t = wp.tile(shape, dtype, tag=tag or name)
            nc.sync.dma_start(out=t[:], in_=D[name][:] if src is None else src)
            return t

        ixs = [ld("ix%d" % T, [P, tok_total[T] // 16], i16) for T in range(3)]
        embT = ld("embT", [HID, VOCAB], bf)
        K1T = ld("K1T", [HID, 3, 96], bf)
        KT2e = ld("KT2e", [97, 3, 128], bf)
        KT3 = ld("KT3", [128, 3, IN_DIM], bf)
        KT4e = ld("KT4e", [75, 3, 128], bf)
        cb3 = ld("cb3", [IN_DIM, 1], f32)

        # M1stack [97, 96]: rows 32t+v = embed[v] @ K1[:,:,t]^T; row 96 = cb1
        # (tap blocks at 32-aligned partition bases; gap rows zeroed so the
        # matmul against zero one-hot rows cannot pick up NaN garbage)
        M1 = wp.tile([97, 96], bf, tag="m1")
        nc.vector.memset(M1[:], 0.0)
        nc.sync.dma_start(out=M1[96:97, :], in_=D["cb1row"][:])
        for t in range(3):
            pm = ps1.tile([VOCAB, 96], f32, space="PSUM", tag="ps1a")
            nc.tensor.matmul(pm[:], embT[:], K1T[:, t, :], start=True,
                             stop=True)
            nc.scalar.copy(M1[32 * t:32 * t + VOCAB, :], pm[:])

        # xs conv buffers: ones rows (matmul-folded bias) + zero guard cols
        x1 = wp.tile([97, 1002], bf, tag="xs1")
        x2 = wp.tile([128, 1002], bf, tag="xs2")
        x3 = wp.tile([75, 1002], bf, tag="xs3")
        nc.sync.dma_start(out=x1[96:97, :], in_=D["ones2"][:])
        nc.sync.dma_start(out=x3[74:75, :], in_=D["ones2"][:])
        for tl, nr in ((x1, 96), (x2, 128), (x3, 74)):
            nc.vector.memset(tl[0:nr, 0:1], 0.0)
            nc.vector.memset(tl[0:nr, 1001:1002], 0.0)

        chunkmax = wp.tile([HID, 2, PPC], f32, tag="chunkmax")
        ident = wp.tile([P, P], f32, tag="ident")
        make_identity(nc, ident[:])

        # ---------------- phase 1a: issue all gathers (gpsimd queues)
        gts = []
        for i, (T, off, ntok) in enumerate(sched):
            g = gpool.tile([P, ntok // P, P], bf, tag="g%d" % i)
            nc.gpsimd.dma_gather(
                out_ap=g[:], in_ap=tabs[T][:],
                idxs_ap=ixs[T][:, off // 16:(off + ntok) // 16],
                num_idxs=ntok, num_idxs_reg=ntok, elem_size=P,
                single_packet=False, queue_num=i % NQ)
            gts.append((T, off, ntok, g))

        def g_slice(T, toff, width):
            # locate the gather tile holding table-T tokens [toff, toff+width)
            for (Tg, off, ntok, g) in gts:
                if Tg == T and off <= toff and toff + width <= off + ntok:
                    b = (toff - off) // P
                    return g[:, b:b + width // P, 0:IN_DIM]
            raise AssertionError("token range not found")

        # ---------------- phase 1b: protein conv stack (dense)
        for p in range(PPC):
            if p % 4 == 0:
                oh = ohp.tile([97, 4 * L], bf, tag="oh%d" % ((p // 4) % 2))
                nc.sync.dma_start(out=oh[:], in_=D["ohS"][p // 4])
            pc = (p % 4) * L
            # conv1: single matmul per chunk (taps stacked, bias row)
            for ci, c0 in enumerate((0, 500)):
                ps = pcv.tile([96, 500], f32, space="PSUM", tag="cps")
                nc.tensor.matmul(ps[:], M1[:], oh[:, pc + c0:pc + c0 + 500],
                                 start=True, stop=True)
                nc.scalar.activation(x1[0:96, 1 + c0:501 + c0], ps[:],
                                     AF.Relu)
            # conv2/3/4: 3 taps, 2 chunks, taps outer (weight reuse)
            for lyr, (KT, xin, nin) in enumerate(
                    ((KT2e, x1, 97), (KT3, x2, 128), (KT4e, x3, 75))):
                pss = [pcv.tile([CHANNELS[lyr + 2], 500], f32, space="PSUM",
                                tag="cps", name="cps%d" % ci)
                       for ci in range(2)]
                for tap in range(3):
                    for ci, c0 in enumerate((0, 500)):
                        nc.tensor.matmul(
                            pss[ci][:], KT[:, tap, :],
                            xin[0:nin, c0 + tap:c0 + tap + 500],
                            start=(tap == 0), stop=(tap == 2))
                for ci, c0 in enumerate((0, 500)):
                    if lyr == 0:    # -> xs2, relu via DVE (bias in psum)
                        nc.vector.tensor_scalar(
                            out=x2[:, 1 + c0:501 + c0], in0=pss[ci][:],
                            scalar1=0.0, scalar2=None, op0=ALU.max)
                    elif lyr == 1:  # -> xs3, relu+bias via ACT
                        nc.scalar.activation(
                            x3[0:IN_DIM, 1 + c0:501 + c0], pss[ci][:],
                            AF.Relu, bias=cb3[:])
                    else:           # conv4: max-pool the chunk
                        nc.vector.tensor_reduce(
                            out=chunkmax[:, ci, p:p + 1], in_=pss[ci][:],
                            axis=AX.X, op=ALU.max)

        # ---------------- phase 2: pmax, edge reduces, GNN chain
        # (wait-until hint orders all phase-2 ops after the conv stream in
        # every engine queue; runtime sync is still semaphore-driven)
        ctx.enter_context(tc.tile_wait_until(1.0))
        W_gc = ld("W_gc", [IN_DIM, HID], bf)
        b_gc = ld("b_gc", [HID, 1], f32)
        W_r2 = ld("W_r2", [HID, HID], bf)
        Wc1 = ld("Wc1", [HID, HID], bf); bc1 = ld("bc1", [HID, 1], f32)
        Wc2 = ld("Wc2", [HID, HID], bf); bc2 = ld("bc2", [HID, 1], f32)
        Wf1 = ld("Wf1_r", [HID, 2, 2 * HID], bf)
        bf1 = ld("bf1_r", [HID, 2, 1], f32)
        Wf2 = ld("Wf2_r", [HID, 2, 1], bf)
        bf2 = ld("bf2", [1, 1], f32)
        rdgi = ld("rdgi", [P, NT], f32)
        Sg = ld("S", [P, NT, GPC], bf)
        B2 = ld("B2", [HID, GPC], f32)
        mxt = wp.tile([HID, PPC], f32, tag="mxt")
        nc.vector.tensor_reduce(out=mxt[:],
                                in_=chunkmax[:].rearrange("p c q -> p q c"),
                                axis=AX.X, op=ALU.max)
        pmax = wp.tile([HID, PPC], bf, tag="pmax")
        nc.scalar.activation(pmax[:], mxt[:], AF.Relu)

        # per-tile segment sums: in-place bf16 tree-adds on the contiguous
        # gather slots, then f32 combine + scale, PE-transpose to bf16 aggT
        def g_block(T, toff):
            for (Tg, off, ntok, g) in gts:
                if Tg == T and off <= toff < off + ntok:
                    return g, (toff - off) // P
            raise AssertionError("token offset not found")

        aggs = []
        with nc.allow_low_precision(reason="bf16 edge sums, tol 2e-2"):
            for t in range(NT):
                agg = aggp.tile([IN_DIM, P], bf, tag="agg%d" % t)
                aggs.append(agg)
                if not live[t]:
                    nc.vector.memset(agg[:], 0.0)
                    continue
                slots = []
                for (T, k, toff) in tile_tabs[t]:
                    g, b = g_block(T, toff)
                    while k > 1:
                        h = k // 2
                        nc.vector.tensor_tensor(
                            out=g[:, b:b + h, :], in0=g[:, b:b + h, :],
                            in1=g[:, b + k - h:b + k, :], op=ALU.add)
                        k -= h
                    slots.append(g[:, b, :])
                if len(slots) == 3:   # fold the rare T2 slot in bf16
                    nc.vector.tensor_tensor(out=slots[0], in0=slots[0],
                                            in1=slots[2], op=ALU.add)
                    slots = slots[:2]
                acc = redp.tile([P, P], f32, tag="red0", name="red")
                if len(slots) == 2:
                    nc.vector.tensor_tensor(out=acc[:], in0=slots[0],
                                            in1=slots[1], op=ALU.add)
                else:
                    nc.vector.tensor_scalar(out=acc[:], in0=slots[0],
                                            scalar1=1.0, scalar2=None,
                                            op0=ALU.mult)
                nc.vector.tensor_scalar_mul(acc[:], acc[:], rdgi[:, t:t + 1])
                tp = pgn.tile([IN_DIM, P], f32, space="PSUM", tag="gps")
                nc.tensor.transpose(tp[:], acc[:, :IN_DIM], ident[:])
                nc.scalar.copy(agg[:], tp[:])

        # GNN chain per tile, accumulating hgT = sum_t x2n_t^T-free S product
        hgps = ps1.tile([HID, GPC], f32, space="PSUM", tag="hgps")
        lt = [t for t in range(NT)]
        for t in lt:
            hps = pgn.tile([HID, P], f32, space="PSUM", tag="gps")
            nc.tensor.matmul(hps[:], W_gc[:], aggs[t][:], start=True,
                             stop=True)
            h = gnp.tile([HID, P], bf, tag="h")
            nc.scalar.activation(h[:], hps[:], AF.Relu, bias=b_gc[:])
            x2ps = pgn.tile([P, HID], f32, space="PSUM", tag="gps")
            nc.tensor.matmul(x2ps[:], h[:], W_r2[:], start=True, stop=True)
            x2n = gnp.tile([P, HID], bf, tag="x2n")
            nc.scalar.copy(x2n[:], x2ps[:])
            nc.tensor.matmul(hgps[:], x2n[:], Sg[:, t, :],
                             start=(t == lt[0]), stop=(t == lt[-1]),
                             skip_group_check=True)
        hgf = wp.tile([HID, GPC], f32, tag="hgf")
        nc.vector.scalar_tensor_tensor(out=hgf[:], in0=hgps[:], scalar=1.0,
                                       in1=B2[:], op0=ALU.mult, op1=ALU.add)
        hg = wp.tile([HID, GPC], bf, tag="hg")
        nc.scalar.activation(hg[:], hgf[:], AF.Relu)
        # compound FC
        c1ps = pgn.tile([HID, GPC], f32, space="PSUM", tag="gps")
        nc.tensor.matmul(c1ps[:], Wc1[:], hg[:], start=True, stop=True)
        cv1 = wp.tile([HID, GPC], bf, tag="cv1")
        nc.scalar.activation(cv1[:], c1ps[:], AF.Relu, bias=bc1[:])
        c2ps = pgn.tile([HID, GPC], f32, space="PSUM", tag="gps")
        nc.tensor.matmul(c2ps[:], Wc2[:], cv1[:], start=True, stop=True)
        cv2 = wp.tile([HID, GPC], bf, tag="cv2")
        nc.scalar.activation(cv2[:], c2ps[:], AF.Relu, bias=bc2[:])
        # CPI head: z = [cv2; pmax]
        zin = [cv2, pmax]
        z2 = []
        for mc in range(2):
            zps = pgn.tile([HID, GPC], f32, space="PSUM", tag="gps")
            for kc in range(2):
                nc.tensor.matmul(zps[:], Wf1[:, kc, mc * HID:(mc + 1) * HID],
                                 zin[kc][:, :GPC], start=(kc == 0),
                                 stop=(kc == 1))
            zt = wp.tile([HID, GPC], bf, tag="z2_%d" % mc)
            nc.scalar.activation(zt[:], zps[:], AF.Relu, bias=bf1[:, mc, :])
            z2.append(zt)
        ops = ps1.tile([1, GPC], f32, space="PSUM", tag="ps1a")
        for kc in range(2):
            nc.tensor.matmul(ops[:], Wf2[:, kc, :], z2[kc][:],
                             start=(kc == 0), stop=(kc == 1))
        ot = wp.tile([1, GPC], f32, tag="ot")
        nc.scalar.activation(ot[:], ops[:], AF.Sigmoid, bias=bf2[:1, :])
        nc.sync.dma_start(out=out_d[:], in_=ot[:])

    nc.compile()
    return nc


def kernel(**inputs):
    shared, percore, meta = _host_prep(inputs)
    nc = _build(shared, meta)
    in_maps = []
    for c in range(NCORES):
        m = dict(shared)
        m.update(percore[c])
        in_maps.append(m)
    res = run_bass_kernel_spmd(nc, in_maps, list(range(NCORES)))
    out = np.concatenate([res.results[c]["out"].reshape(GPC)
                          for c in range(NCORES)])
    return out.reshape(B, 1).astype(np.float32)


if __name__ == "__main__":
    sys.path.insert(0, "/root/problem")
    import jax
    import reference
    with jax.default_device(jax.devices("cpu")[0]):
        inputs = {k: np.asarray(v) for k, v in reference.setup_inputs().items()}
        exp = np.asarray(reference.reference(**inputs))
    got = kernel(**inputs)
    err = np.abs(got - exp).max()
    rel = err / max(np.abs(exp).max(), 1e-9)
    print("max abs err:", err, " rel:", rel)
